# revision 1
# baseline (speedup 1.0000x reference)
"""Single-head causal self-attention (B=4, S=4096, D=512) on 8 trn2 NeuronCores.

Sharding: 2 cores per batch element. Each core handles ALL queries of its
batch but only the even- or odd-indexed 128-row KV tiles (flash-decoding
style KV-parity split). This keeps the SPMD program identical across cores,
perfectly load-balances the causal triangle, and wastes no tiles.

Each core emits unnormalized partial outputs po = (sum_k exp(s) * v) @ Wo^T
and partial row-sums l. Host combines: out = (po0 + po1) / (l0 + l1) + bo.
Scores/sqrt(D) are bounded (~N(0, 0.33)) for well-scaled inputs, so exp
without max-subtraction is safe; softmax is shift-invariant so the result
matches the reference.

Dataflow is fully transposed (x and weights passed pre-transposed, bf16) so
no on-device transposes are needed:
  Q^T[d,q]  = (WqT chunks)^T @ xT          (lhsT=WqT[e,d], rhs=xT[e,q])
  K^T[d,k]  = same with WkT, x-local
  V[k,d]    = (x-local chunks)^T @ WvT     (lhsT=xT[e,k], rhs=WvT[e,d])
  S^T[k,q]  = (K^T chunks)^T @ Q^T         accum over d
  P^T       = exp(S^T * scale)             (ScalarE, PSUM->SBUF bf16)
  l[1,q]    = ones^T @ P^T                 accum over k tiles
  A^T[d,q]  = (V chunks)^T @ P^T           accum over k tiles
  po[q,e]   = (A^T chunks)^T @ WoT         accum over d
"""

import numpy as np
import ml_dtypes

import concourse.bass as bass
import concourse.tile as tile
from concourse import bacc, mybir
from concourse import bass_utils

B, S, D = 4, 4096, 512
TS = 128              # kv tile rows
QB = 512              # query block
NQB = S // QB         # 8 query blocks
NLT = S // TS // 2    # 16 local kv tiles per core
NDC = D // 128        # 4 chunks of d
SL = S // 2           # 2048 local kv rows
SCALE = 1.0 / float(np.sqrt(D))
BF16 = mybir.dt.bfloat16
F32 = mybir.dt.float32
N_CORES = 8


def build_kernel(repeat=1):
    nc = bacc.Bacc("TRN2", target_bir_lowering=False, debug=False)

    xT = nc.dram_tensor("xT", [D, S], BF16, kind="ExternalInput").ap()
    xfl = nc.dram_tensor("xfl", [D, SL], BF16, kind="ExternalInput").ap()
    wqT = nc.dram_tensor("wqT", [D, D], BF16, kind="ExternalInput").ap()
    wkT = nc.dram_tensor("wkT", [D, D], BF16, kind="ExternalInput").ap()
    wvT = nc.dram_tensor("wvT", [D, D], BF16, kind="ExternalInput").ap()
    woT = nc.dram_tensor("woT", [D, D], BF16, kind="ExternalInput").ap()
    masks = nc.dram_tensor("masks", [2 * TS, QB], BF16, kind="ExternalInput").ap()
    po = nc.dram_tensor("po", [S, D], F32, kind="ExternalOutput").ap()
    lrow = nc.dram_tensor("lrow", [1, S], F32, kind="ExternalOutput").ap()

    with tile.TileContext(nc) as tc:
        with tc.tile_pool(name="persist", bufs=1) as P:
            xT_sb = [P.tile([TS, S], BF16, tag=f"xT{e}", name=f"xT{e}") for e in range(NDC)]
            xfl_sb = [P.tile([TS, SL], BF16, tag=f"xfl{e}", name=f"xfl{e}") for e in range(NDC)]
            w_sb = {
                wname: [P.tile([TS, D], BF16, tag=f"w{wname}{e}", name=f"w{wname}{e}")
                        for e in range(NDC)]
                for wname in ("q", "k", "v", "o")
            }
            # DMA issue order = first-consumption order: V-proj needs xfl+wvT
            # first, then wkT (K-proj), then xT+wqT (Q-proj), then woT.
            # xfl/xT are split into 512-col pieces so the first matmuls can
            # start as soon as the leading columns of every e-chunk land.
            for cb in range(SL // QB):
                for e in range(NDC):
                    nc.sync.dma_start(xfl_sb[e][:, cb * QB:(cb + 1) * QB],
                                      xfl[e * TS:(e + 1) * TS, cb * QB:(cb + 1) * QB])
                if cb == 0:
                    for e in range(NDC):
                        nc.sync.dma_start(w_sb["v"][e][:], wvT[e * TS:(e + 1) * TS, :])
                if cb == 1:
                    for e in range(NDC):
                        nc.sync.dma_start(w_sb["k"][e][:], wkT[e * TS:(e + 1) * TS, :])
            for cb in range(S // QB):
                for e in range(NDC):
                    nc.sync.dma_start(xT_sb[e][:, cb * QB:(cb + 1) * QB],
                                      xT[e * TS:(e + 1) * TS, cb * QB:(cb + 1) * QB])
                if cb == 0:
                    for e in range(NDC):
                        nc.sync.dma_start(w_sb["q"][e][:], wqT[e * TS:(e + 1) * TS, :])
                if cb == 1:
                    for e in range(NDC):
                        nc.sync.dma_start(w_sb["o"][e][:], woT[e * TS:(e + 1) * TS, :])
            mask_sb = [P.tile([TS, QB], BF16, tag=f"mask{c}", name=f"mask{c}") for c in range(2)]
            for c in range(2):
                nc.sync.dma_start(mask_sb[c][:], masks[c * TS:(c + 1) * TS, :])
            ones_sb = P.tile([TS, 1], BF16, tag="ones", name="ones")
            nc.gpsimd.memset(ones_sb[:], 1.0)

            QT_sb = [P.tile([TS, S], BF16, tag=f"QT{dc}", name=f"QT{dc}") for dc in range(NDC)]
            KT_sb = [P.tile([TS, SL], BF16, tag=f"KT{dc}", name=f"KT{dc}") for dc in range(NDC)]
            V_sb = [P.tile([TS, D], BF16, tag=f"V{kt}", name=f"V{kt}") for kt in range(NLT)]

            from contextlib import ExitStack
            with ExitStack() as rep_ctx:
                if repeat > 1:
                    rep_ctx.enter_context(tc.For_i(0, repeat, 1))
                # ---- projections ----
                with tc.tile_pool(name="proj_ps", bufs=4, space="PSUM") as PP:
                    # V[k,d]: lhsT = xfl[e, k-chunk], rhs = WvT[e, :]
                    for kt in range(NLT):
                        ps = PP.tile([TS, D], F32, tag="ps", name="ps_v")
                        for e in range(NDC):
                            nc.tensor.matmul(
                                ps[:], xfl_sb[e][:, kt * TS:(kt + 1) * TS], w_sb["v"][e][:],
                                start=(e == 0), stop=(e == NDC - 1))
                        nc.vector.tensor_copy(V_sb[kt][:], ps[:])
                    # K^T[d,k]: lhsT = WkT[e, d-chunk], rhs = xfl[e, colblock]
                    for dc in range(NDC):
                        for cb in range(SL // QB):
                            ps = PP.tile([TS, QB], F32, tag="ps", name="ps_p")
                            for e in range(NDC):
                                nc.tensor.matmul(
                                    ps[:], w_sb["k"][e][:, dc * TS:(dc + 1) * TS],
                                    xfl_sb[e][:, cb * QB:(cb + 1) * QB],
                                    start=(e == 0), stop=(e == NDC - 1))
                            nc.vector.tensor_copy(KT_sb[dc][:, cb * QB:(cb + 1) * QB], ps[:])
                    # Q^T[d,q]: lhsT = WqT[e, d-chunk], rhs = xT[e, colblock]
                    for dc in range(NDC):
                        for cb in range(S // QB):
                            ps = PP.tile([TS, QB], F32, tag="ps", name="ps_p")
                            for e in range(NDC):
                                nc.tensor.matmul(
                                    ps[:], w_sb["q"][e][:, dc * TS:(dc + 1) * TS],
                                    xT_sb[e][:, cb * QB:(cb + 1) * QB],
                                    start=(e == 0), stop=(e == NDC - 1))
                            nc.vector.tensor_copy(QT_sb[dc][:, cb * QB:(cb + 1) * QB], ps[:])

                # ---- attention + output projection, per query block ----
                with tc.tile_pool(name="st_ps", bufs=2, space="PSUM") as STP, \
                     tc.tile_pool(name="attn_ps", bufs=1, space="PSUM") as ATP, \
                     tc.tile_pool(name="l_ps", bufs=1, space="PSUM") as LP, \
                     tc.tile_pool(name="po_ps", bufs=1, space="PSUM") as POP, \
                     tc.tile_pool(name="p_sb", bufs=6) as PSB, \
                     tc.tile_pool(name="o_sb", bufs=3) as OSB:
                    for j in range(NQB):
                        nlt = 2 * j + 2
                        qcol = slice(j * QB, (j + 1) * QB)
                        attn_ps = [ATP.tile([TS, QB], F32, tag=f"attn{dc}", name=f"attn{dc}") for dc in range(NDC)]
                        l_ps = LP.tile([1, QB], F32, tag="l", name="l")
                        for lt in range(nlt):
                            st = STP.tile([TS, QB], F32, tag="st", name="st")
                            for dc in range(NDC):
                                nc.tensor.matmul(
                                    st[:], KT_sb[dc][:, lt * TS:(lt + 1) * TS], QT_sb[dc][:, qcol],
                                    start=(dc == 0), stop=(dc == NDC - 1))
                            p = PSB.tile([TS, QB], BF16, tag="p", name="p")
                            nc.scalar.activation(
                                p[:], st[:], mybir.ActivationFunctionType.Exp, scale=SCALE)
                            if lt >= 2 * j:
                                nc.vector.tensor_mul(p[:], p[:], mask_sb[lt - 2 * j][:])
                            nc.tensor.matmul(l_ps[:], ones_sb[:], p[:],
                                             start=(lt == 0), stop=(lt == nlt - 1))
                            for dc in range(NDC):
                                nc.tensor.matmul(
                                    attn_ps[dc][:], V_sb[lt][:, dc * TS:(dc + 1) * TS], p[:],
                                    start=(lt == 0), stop=(lt == nlt - 1))
                        l_sb = OSB.tile([1, QB], F32, tag="l_sb", name="l_sb")
                        nc.vector.tensor_copy(l_sb[:], l_ps[:])
                        nc.sync.dma_start(lrow[0:1, qcol], l_sb[:])
                        attn_sb = [OSB.tile([TS, QB], BF16, tag=f"attn_sb{dc}", name=f"attn_sb{dc}") for dc in range(NDC)]
                        for half in range(2):
                            hs = slice(half * (QB // 2), (half + 1) * (QB // 2))
                            for dc in range(NDC):
                                nc.vector.tensor_copy(attn_sb[dc][:, hs], attn_ps[dc][:, hs])
                        for qc in range(QB // TS):
                            po_ps = POP.tile([TS, D], F32, tag="po", name="po_ps_t")
                            for dc in range(NDC):
                                nc.tensor.matmul(
                                    po_ps[:], attn_sb[dc][:, qc * TS:(qc + 1) * TS], w_sb["o"][dc][:],
                                    start=(dc == 0), stop=(dc == NDC - 1))
                            po_sb = OSB.tile([TS, D], F32, tag="po_sb", name="po_sb")
                            nc.vector.tensor_copy(po_sb[:], po_ps[:])
                            r0 = j * QB + qc * TS
                            nc.sync.dma_start(po[r0:r0 + TS, :], po_sb[:])
    nc.compile()
    return nc


_cache = {}


def _make_masks(h):
    m = np.zeros((2 * TS, QB), dtype=np.float32)
    k_r = np.arange(TS)[:, None]
    q_r = np.arange(QB)[None, :]
    for c in range(2):
        m[c * TS:(c + 1) * TS] = (q_r >= 128 * (2 * c + h) + k_r)
    return m.astype(ml_dtypes.bfloat16)


def kernel(x, Wq, Wk, Wv, Wo, bo):
    bf = ml_dtypes.bfloat16
    x = np.asarray(x, dtype=np.float32)
    Wq, Wk, Wv, Wo, bo = (np.asarray(a, dtype=np.float32) for a in (Wq, Wk, Wv, Wo, bo))
    if "nc" not in _cache:
        _cache["nc"] = build_kernel()
    nc = _cache["nc"]

    wqT = np.ascontiguousarray(Wq.T).astype(bf)
    wkT = np.ascontiguousarray(Wk.T).astype(bf)
    wvT = np.ascontiguousarray(Wv.T).astype(bf)
    woT = np.ascontiguousarray(Wo.T).astype(bf)
    mask_h = [_make_masks(0), _make_masks(1)]

    # local kv columns for parity h: 128-col tiles with global tile index % 2 == h
    col_idx = {}
    for h in range(2):
        tiles = [np.arange(TS * (2 * lt + h), TS * (2 * lt + h) + TS) for lt in range(NLT)]
        col_idx[h] = np.concatenate(tiles)

    in_maps = []
    for core in range(N_CORES):
        b, h = core // 2, core % 2
        xTb = np.ascontiguousarray(x[b].T).astype(bf)     # [D, S]
        xflb = np.ascontiguousarray(xTb[:, col_idx[h]])
        in_maps.append({
            "xT": xTb, "xfl": xflb,
            "wqT": wqT, "wkT": wkT, "wvT": wvT, "woT": woT,
            "masks": mask_h[h],
        })

    global _last_in_maps
    _last_in_maps = in_maps
    res = bass_utils.run_bass_kernel_spmd(nc, in_maps, core_ids=list(range(N_CORES)))

    out = np.zeros((B, S, D), dtype=np.float32)
    for b in range(B):
        r0, r1 = res.results[2 * b], res.results[2 * b + 1]
        l = (r0["lrow"] + r1["lrow"]).reshape(S, 1)
        out[b] = (r0["po"] + r1["po"]) / l + bo.astype(np.float32)
    return out



# revision 2
# speedup vs baseline: 1.0765x; 1.0765x over previous
"""Single-head causal self-attention (B=4, S=4096, D=512) on 8 trn2 NeuronCores.

Sharding: 2 cores per batch element. Each core handles ALL queries of its
batch but only the even- or odd-indexed 128-row KV tiles (flash-decoding
style KV-parity split). This keeps the SPMD program identical across cores,
perfectly load-balances the causal triangle, and wastes no tiles.

Weight folding (host-side, f32): scores = x Wq^T Wk x^T = x G x^T with
G = Wq^T @ Wk, and the output projection is folded into the values:
Vt = V Wo^T = x (Wv^T Wo^T) = x H. The device then only computes
  R^T = (x G)^T           lhsT=G chunks, rhs=x^T          [b, q]
  Vt[k,e]                 lhsT=x^T local chunks, rhs=H    [k, e]
  S^T[k,q] = sum_b x^T[b,k]^T R[b,q]   (lhsT = x^T local) accum over b
  P^T      = exp(S^T * scale)          (ScalarE, PSUM->SBUF bf16)
  l[1,q]   = ones^T @ P^T              accum over k tiles
  poT[e,q] = sum_k Vt[k,e]^T P^T[k,q]  accum over k tiles  (= final proj!)
This removes the K projection and the entire output projection from the PE.

Each core emits unnormalized partial outputs poT (already in output space)
and partial row-sums l. Host combines: out = ((poT0 + poT1) / (l0 + l1)).T
+ bo. Scores/sqrt(D) are bounded (~N(0, 0.33)) for well-scaled inputs, so
exp without max-subtraction is safe; softmax is shift-invariant so the
result matches the reference.
"""

import numpy as np
import ml_dtypes

import concourse.bass as bass
import concourse.tile as tile
from concourse import bacc, mybir
from concourse import bass_utils

B, S, D = 4, 4096, 512
TS = 128              # kv tile rows
QB = 512              # query block
NQB = S // QB         # 8 query blocks
NLT = S // TS // 2    # 16 local kv tiles per core
NDC = D // 128        # 4 chunks of d
SL = S // 2           # 2048 local kv rows
SCALE = 1.0 / float(np.sqrt(D))
BF16 = mybir.dt.bfloat16
F32 = mybir.dt.float32
N_CORES = 8


def build_kernel(repeat=1):
    nc = bacc.Bacc("TRN2", target_bir_lowering=False, debug=False)

    xT = nc.dram_tensor("xT", [D, S], BF16, kind="ExternalInput").ap()
    xfl = nc.dram_tensor("xfl", [D, SL], BF16, kind="ExternalInput").ap()
    g = nc.dram_tensor("g", [D, D], BF16, kind="ExternalInput").ap()
    h = nc.dram_tensor("h", [D, D], BF16, kind="ExternalInput").ap()
    masks = nc.dram_tensor("masks", [2 * TS, QB], BF16, kind="ExternalInput").ap()
    poT = nc.dram_tensor("poT", [D, S], F32, kind="ExternalOutput").ap()
    lrow = nc.dram_tensor("lrow", [1, S], F32, kind="ExternalOutput").ap()

    with tile.TileContext(nc) as tc:
        with tc.tile_pool(name="persist", bufs=1) as P:
            xT_sb = [P.tile([TS, S], BF16, tag=f"xT{e}", name=f"xT{e}") for e in range(NDC)]
            xfl_sb = [P.tile([TS, SL], BF16, tag=f"xfl{e}", name=f"xfl{e}") for e in range(NDC)]
            g_sb = [P.tile([TS, D], BF16, tag=f"g{e}", name=f"g{e}") for e in range(NDC)]
            h_sb = [P.tile([TS, D], BF16, tag=f"h{e}", name=f"h{e}") for e in range(NDC)]
            # DMA issue order = first-consumption order: Vt-proj needs xfl+h
            # first, then R-proj needs xT+g. x is split into 512-col pieces so
            # the first matmuls can start as soon as the leading columns of
            # every e-chunk land.
            for cb in range(SL // QB):
                for e in range(NDC):
                    nc.sync.dma_start(xfl_sb[e][:, cb * QB:(cb + 1) * QB],
                                      xfl[e * TS:(e + 1) * TS, cb * QB:(cb + 1) * QB])
                if cb == 0:
                    for e in range(NDC):
                        nc.sync.dma_start(h_sb[e][:], h[e * TS:(e + 1) * TS, :])
                if cb == 1:
                    for e in range(NDC):
                        nc.sync.dma_start(g_sb[e][:], g[e * TS:(e + 1) * TS, :])
            for cb in range(S // QB):
                for e in range(NDC):
                    nc.sync.dma_start(xT_sb[e][:, cb * QB:(cb + 1) * QB],
                                      xT[e * TS:(e + 1) * TS, cb * QB:(cb + 1) * QB])
            mask_sb = [P.tile([TS, QB], BF16, tag=f"mask{c}", name=f"mask{c}") for c in range(2)]
            for c in range(2):
                nc.sync.dma_start(mask_sb[c][:], masks[c * TS:(c + 1) * TS, :])
            ones_sb = P.tile([TS, 1], BF16, tag="ones", name="ones")
            nc.gpsimd.memset(ones_sb[:], 1.0)

            R_sb = [P.tile([TS, S], BF16, tag=f"R{dc}", name=f"R{dc}") for dc in range(NDC)]
            Vt_sb = [P.tile([TS, D], BF16, tag=f"Vt{kt}", name=f"Vt{kt}") for kt in range(NLT)]

            from contextlib import ExitStack
            with ExitStack() as rep_ctx:
                if repeat > 1:
                    rep_ctx.enter_context(tc.For_i(0, repeat, 1))
                # ---- projections ----
                with tc.tile_pool(name="proj_ps", bufs=4, space="PSUM") as PP:
                    # Vt[k,e]: lhsT = xfl[c, k-chunk], rhs = H[c, :]
                    for kt in range(NLT):
                        ps = PP.tile([TS, D], F32, tag="ps", name="ps_v")
                        for e in range(NDC):
                            nc.tensor.matmul(
                                ps[:], xfl_sb[e][:, kt * TS:(kt + 1) * TS], h_sb[e][:],
                                start=(e == 0), stop=(e == NDC - 1))
                        nc.vector.tensor_copy(Vt_sb[kt][:], ps[:])
                    # R[b,q] = (xG)^T: lhsT = G[a, b-chunk], rhs = xT[a, colblock]
                    for dc in range(NDC):
                        for cb in range(S // QB):
                            ps = PP.tile([TS, QB], F32, tag="ps", name="ps_p")
                            for e in range(NDC):
                                nc.tensor.matmul(
                                    ps[:], g_sb[e][:, dc * TS:(dc + 1) * TS],
                                    xT_sb[e][:, cb * QB:(cb + 1) * QB],
                                    start=(e == 0), stop=(e == NDC - 1))
                            nc.vector.tensor_copy(R_sb[dc][:, cb * QB:(cb + 1) * QB], ps[:])

                # ---- attention (directly in output space), per query block ----
                with tc.tile_pool(name="st_ps", bufs=2, space="PSUM") as STP, \
                     tc.tile_pool(name="attn_ps", bufs=1, space="PSUM") as ATP, \
                     tc.tile_pool(name="l_ps", bufs=1, space="PSUM") as LP, \
                     tc.tile_pool(name="p_sb", bufs=6) as PSB, \
                     tc.tile_pool(name="o_sb", bufs=3) as OSB:
                    for j in range(NQB):
                        nlt = 2 * j + 2
                        qcol = slice(j * QB, (j + 1) * QB)
                        attn_ps = [ATP.tile([TS, QB], F32, tag=f"attn{dc}", name=f"attn{dc}") for dc in range(NDC)]
                        l_ps = LP.tile([1, QB], F32, tag="l", name="l")
                        for lt in range(nlt):
                            st = STP.tile([TS, QB], F32, tag="st", name="st")
                            for dc in range(NDC):
                                nc.tensor.matmul(
                                    st[:], xfl_sb[dc][:, lt * TS:(lt + 1) * TS], R_sb[dc][:, qcol],
                                    start=(dc == 0), stop=(dc == NDC - 1))
                            p = PSB.tile([TS, QB], BF16, tag="p", name="p")
                            nc.scalar.activation(
                                p[:], st[:], mybir.ActivationFunctionType.Exp, scale=SCALE)
                            if lt >= 2 * j:
                                nc.vector.tensor_mul(p[:], p[:], mask_sb[lt - 2 * j][:])
                            nc.tensor.matmul(l_ps[:], ones_sb[:], p[:],
                                             start=(lt == 0), stop=(lt == nlt - 1))
                            for dc in range(NDC):
                                nc.tensor.matmul(
                                    attn_ps[dc][:], Vt_sb[lt][:, dc * TS:(dc + 1) * TS], p[:],
                                    start=(lt == 0), stop=(lt == nlt - 1))
                        l_sb = OSB.tile([1, QB], F32, tag="l_sb", name="l_sb")
                        nc.vector.tensor_copy(l_sb[:], l_ps[:])
                        nc.sync.dma_start(lrow[0:1, qcol], l_sb[:])
                        for dc in range(NDC):
                            po_sb = OSB.tile([TS, QB], F32, tag=f"po_sb{dc}", name=f"po_sb{dc}")
                            for half in range(2):
                                hs = slice(half * (QB // 2), (half + 1) * (QB // 2))
                                nc.vector.tensor_copy(po_sb[:, hs], attn_ps[dc][:, hs])
                            nc.sync.dma_start(poT[dc * TS:(dc + 1) * TS, qcol], po_sb[:])
    nc.compile()
    return nc


_cache = {}


def _make_masks(h):
    m = np.zeros((2 * TS, QB), dtype=np.float32)
    k_r = np.arange(TS)[:, None]
    q_r = np.arange(QB)[None, :]
    for c in range(2):
        m[c * TS:(c + 1) * TS] = (q_r >= 128 * (2 * c + h) + k_r)
    return m.astype(ml_dtypes.bfloat16)


def kernel(x, Wq, Wk, Wv, Wo, bo):
    bf = ml_dtypes.bfloat16
    x = np.asarray(x, dtype=np.float32)
    Wq, Wk, Wv, Wo, bo = (np.asarray(a, dtype=np.float32) for a in (Wq, Wk, Wv, Wo, bo))
    if "nc" not in _cache:
        _cache["nc"] = build_kernel()
    nc = _cache["nc"]

    # fold the projections: scores = x G x^T, Vt = x H (= V Wo^T)
    G = np.ascontiguousarray(Wq.T @ Wk).astype(bf)
    H = np.ascontiguousarray(Wv.T @ Wo.T).astype(bf)
    mask_h = [_make_masks(0), _make_masks(1)]

    # local kv columns for parity h: 128-col tiles with global tile index % 2 == h
    col_idx = {}
    for h in range(2):
        tiles = [np.arange(TS * (2 * lt + h), TS * (2 * lt + h) + TS) for lt in range(NLT)]
        col_idx[h] = np.concatenate(tiles)

    in_maps = []
    for core in range(N_CORES):
        b, h = core // 2, core % 2
        xTb = np.ascontiguousarray(x[b].T).astype(bf)     # [D, S]
        xflb = np.ascontiguousarray(xTb[:, col_idx[h]])
        in_maps.append({
            "xT": xTb, "xfl": xflb,
            "g": G, "h": H,
            "masks": mask_h[h],
        })

    global _last_in_maps
    _last_in_maps = in_maps
    res = bass_utils.run_bass_kernel_spmd(nc, in_maps, core_ids=list(range(N_CORES)))

    out = np.zeros((B, S, D), dtype=np.float32)
    for b in range(B):
        r0, r1 = res.results[2 * b], res.results[2 * b + 1]
        l = (r0["lrow"] + r1["lrow"]).reshape(1, S)
        out[b] = ((r0["poT"] + r1["poT"]) / l).T + bo.astype(np.float32)
    return out
